# revision 1
# baseline (speedup 1.0000x reference)
"""Trainium2 Bass kernel for BlockGivensRotation (w @ R, block-diagonal).

The reference applies, per 128-column block of w, 8 sequential sweeps of 127
adjacent-plane Givens rotations.  The composition of all 1016 rotations of a
block is a fixed 128x128 orthogonal matrix R_nb that depends only on `angles`,
so the whole op is `out[:, nb*128:(nb+1)*128] = w[:, nb*128:(nb+1)*128] @ R_nb`
- a block-diagonal matmul, ideal for the tensor engine.

Host side: compose R (tiny: 64x128x128, built in f64 from the 65K angles).
Device side (per core, 8-way row sharding of w): stream w.T tiles from DRAM,
matmul with the per-block stationary R, write out.T tiles back.  w is fed
transposed so that the contraction dim (block columns) lies on SBUF partitions
with fully contiguous DMA; the host transposes shards in/out.
"""

import numpy as np

import concourse.bacc as bacc
import concourse.mybir as mybir
import concourse.tile as tile
from concourse.bass_utils import run_bass_kernel_spmd

O = 8192          # output rows (w rows)
IN_F = 8192       # w cols
B = 128           # Givens block size
NB = IN_F // B    # 64 blocks
S = 8             # sweeps
NCORES = 8
ROWS = O // NCORES  # 1024 rows of w per core
F32 = mybir.dt.float32


def _build_rotation_matrices(angles: np.ndarray) -> np.ndarray:
    """Compose the sweeps of adjacent Givens rotations into one 128x128
    matrix per block by applying the reference recurrence to the identity
    (in float64, rounded once to f32)."""
    nb, s, bm1 = angles.shape
    b = bm1 + 1
    ang = np.asarray(angles, dtype=np.float64)
    c = np.cos(ang)
    sn = np.sin(ang)
    R = np.broadcast_to(np.eye(b), (nb, b, b)).copy()  # [NB, basis row, col]
    for sweep in range(s):
        cs, ss = c[:, sweep, :], sn[:, sweep, :]
        carry = R[:, :, 0].copy()
        for i in range(bm1):
            col_j = R[:, :, i + 1]
            ci = cs[:, i][:, None]
            si = ss[:, i][:, None]
            R[:, :, i] = ci * carry - si * col_j
            carry = si * carry + ci * col_j
        R[:, :, b - 1] = carry
    return R.astype(np.float32)


def _build_bass(rows=ROWS, nb=NB, bpt=4, ncores=NCORES):
    """Per-core program: out_t[nb*B+c', r] = sum_c R[nb][c, c'] * wt[nb*B+c, r].

    rows: w rows handled by this core; nb: number of 128-col blocks;
    bpt: blocks per DMA tile (outer loop granularity).
    """
    nc = bacc.Bacc(
        "TRN2", target_bir_lowering=False, debug=False, num_devices=ncores
    )
    wt = nc.dram_tensor("wt", [nb * B, rows], F32, kind="ExternalInput")
    r = nc.dram_tensor("r", [B, nb * B], F32, kind="ExternalInput")
    out_t = nc.dram_tensor("out_t", [nb * B, rows], F32, kind="ExternalOutput")

    nt = nb // bpt
    hchunks = rows // 512 if rows >= 512 else 1
    hsize = min(rows, 512)

    with tile.TileContext(nc) as tc:
        with (
            tc.tile_pool(name="rconst", bufs=1) as rp,
            tc.tile_pool(name="wtp", bufs=3) as wtp,
            tc.tile_pool(name="outp", bufs=3) as outp,
            tc.tile_pool(name="ps", bufs=8, space="PSUM") as psp,
        ):
            r_sb = rp.tile([B, nb * B], F32)
            nc.sync.dma_start(r_sb[:], r[:])
            for t in range(nt):
                wt_tile = wtp.tile([B, bpt, rows], F32)
                nc.sync.dma_start(
                    wt_tile[:],
                    wt[t * bpt * B : (t + 1) * bpt * B, :].rearrange(
                        "(b p) n -> p b n", p=B
                    ),
                )
                out_tile = outp.tile([B, bpt, rows], F32)
                for bi in range(bpt):
                    blk = t * bpt + bi
                    for h in range(hchunks):
                        ps = psp.tile([B, hsize], F32)
                        nc.tensor.matmul(
                            ps[:],
                            r_sb[:, blk * B : (blk + 1) * B],
                            wt_tile[:, bi, h * hsize : (h + 1) * hsize],
                            start=True,
                            stop=True,
                        )
                        nc.vector.tensor_copy(
                            out_tile[:, bi, h * hsize : (h + 1) * hsize], ps[:]
                        )
                nc.sync.dma_start(
                    out_t[t * bpt * B : (t + 1) * bpt * B, :].rearrange(
                        "(b p) n -> p b n", p=B
                    ),
                    out_tile[:],
                )
    nc.compile()
    return nc


def kernel_impl(w, angles, trace=False, **spmd_kwargs):
    w = np.asarray(w)
    Rm = _build_rotation_matrices(np.asarray(angles))
    # r_host[c, nb*B + c'] = R[nb][c, c']  (contiguous per SBUF partition c)
    r_host = np.ascontiguousarray(Rm.transpose(1, 0, 2)).reshape(B, NB * B)
    nc = _build_bass()
    in_maps = [
        {"wt": w[i * ROWS : (i + 1) * ROWS, :].T, "r": r_host}
        for i in range(NCORES)
    ]
    res = run_bass_kernel_spmd(
        nc, in_maps, core_ids=list(range(NCORES)), trace=trace, **spmd_kwargs
    )
    out = np.empty((O, IN_F), dtype=np.float32)
    for i in range(NCORES):
        out[i * ROWS : (i + 1) * ROWS, :] = res.results[i]["out_t"].T
    return out, res


def kernel(w, angles):
    out, _ = kernel_impl(w, angles, trace=False)
    return out


# revision 8
# speedup vs baseline: 1.1698x; 1.1698x over previous
"""Trainium2 Bass kernel for BlockGivensRotation (w @ R, block-diagonal).

The reference applies, per 128-column block of w, 8 sequential sweeps of 127
adjacent-plane Givens rotations.  The composition of all 1016 rotations of a
block is a fixed 128x128 orthogonal matrix R_nb that depends only on `angles`,
so the whole op is `out[:, nb*128:(nb+1)*128] = w[:, nb*128:(nb+1)*128] @ R_nb`
- a block-diagonal matmul, ideal for the tensor engine.

Host side: compose R (tiny: 64x128x128, built in f64 from the 65K angles).
Device side (per core, 8-way row sharding of w): stream w.T tiles from DRAM,
matmul with the per-block stationary R (full fp32), write out.T tiles back.
w is fed transposed so that the contraction dim (block columns) lies on SBUF
partitions with fully contiguous DMA; the host transposes shards in/out.

The kernel is DMA-bound (64 MB of mandatory HBM I/O per core at ~360 GB/s),
so the structure keeps the DMA queues saturated: per-block 512 KB loads and
stores on the two HWDGE rings, R loaded in chunks so the first matmul starts
~2 us in, and deep tile pools so the PE never starves (HAM stays warm).
"""

import numpy as np

import concourse.bacc as bacc
import concourse.mybir as mybir
import concourse.tile as tile
from concourse.bass_utils import run_bass_kernel_spmd

O = 8192          # output rows (w rows)
IN_F = 8192       # w cols
B = 128           # Givens block size
NB = IN_F // B    # 64 blocks
S = 8             # sweeps
NCORES = 8
ROWS = O // NCORES  # 1024 rows of w per core
F32 = mybir.dt.float32


def _build_rotation_matrices(angles: np.ndarray) -> np.ndarray:
    """Compose the sweeps of adjacent Givens rotations into one 128x128
    matrix per block by applying the reference recurrence to the identity
    (in float64, rounded once to f32)."""
    nb, s, bm1 = angles.shape
    b = bm1 + 1
    ang = np.asarray(angles, dtype=np.float64)
    c = np.cos(ang)
    sn = np.sin(ang)
    R = np.broadcast_to(np.eye(b), (nb, b, b)).copy()  # [NB, basis row, col]
    for sweep in range(s):
        cs, ss = c[:, sweep, :], sn[:, sweep, :]
        carry = R[:, :, 0].copy()
        for i in range(bm1):
            col_j = R[:, :, i + 1]
            ci = cs[:, i][:, None]
            si = ss[:, i][:, None]
            R[:, :, i] = ci * carry - si * col_j
            carry = si * carry + ci * col_j
        R[:, :, b - 1] = carry
    return R.astype(np.float32)


def _build_bass(
    rows=ROWS,
    nb=NB,
    ncores=NCORES,
    mm_f32r=False,
    wt_bufs=10,
    out_bufs=8,
    r_chunk=4,
):
    """Per-core program: out_t[blk*B+c', r] = sum_c R[blk][c, c'] * wt[blk*B+c, r].

    rows: w rows handled by this core; nb: number of 128-col blocks;
    mm_f32r: run the matmuls in float32r (single-pass, ~1e-4 rel err);
    wt_bufs/out_bufs: pipeline depth of the w / out tile pools;
    r_chunk: blocks of R per load chunk.
    """
    mm_dt = mybir.dt.float32r if mm_f32r else F32
    nc = bacc.Bacc(
        "TRN2", target_bir_lowering=False, debug=False, num_devices=ncores
    )
    wt = nc.dram_tensor("wt", [nb * B, rows], mm_dt, kind="ExternalInput")
    r = nc.dram_tensor("r", [B, nb * B], mm_dt, kind="ExternalInput")
    out_t = nc.dram_tensor("out_t", [nb * B, rows], F32, kind="ExternalOutput")

    hs = min(rows, 512)   # moving free-dim per matmul (fp32 max 512)
    hc = rows // hs

    with tile.TileContext(nc) as tc:
        with (
            tc.tile_pool(name="rp", bufs=1) as rp,
            tc.tile_pool(name="wtp", bufs=wt_bufs) as wtp,
            tc.tile_pool(name="outp", bufs=out_bufs) as outp,
            tc.tile_pool(name="psp", bufs=8, space="PSUM") as psp,
        ):
            r_tiles = []
            for blk in range(nb):
                tchunk, toff = divmod(blk, r_chunk)
                if toff == 0:
                    csz = min(r_chunk, nb - blk)
                    rt = rp.tile([B, csz * B], mm_dt, tag=f"rchunk{tchunk}")
                    nc.sync.dma_start(
                        rt[:], r[:, blk * B : (blk + csz) * B]
                    )
                    r_tiles.append(rt)
                wt_tile = wtp.tile([B, rows], mm_dt)
                nc.sync.dma_start(wt_tile[:], wt[blk * B : (blk + 1) * B, :])
                out_tile = outp.tile([B, rows], F32)
                r_ap = r_tiles[tchunk][:, toff * B : (toff + 1) * B]
                for h in range(hc):
                    ps = psp.tile([B, hs], F32)
                    nc.tensor.matmul(
                        ps[:],
                        r_ap,
                        wt_tile[:, h * hs : (h + 1) * hs],
                        start=True,
                        stop=True,
                    )
                    nc.vector.tensor_copy(out_tile[:, h * hs : (h + 1) * hs], ps[:])
                # out-stores ride the second HWDGE ring (ACT)
                nc.scalar.dma_start(out_t[blk * B : (blk + 1) * B, :], out_tile[:])
    nc.compile()
    return nc


def kernel_impl(w, angles, trace=False, mm_f32r=False, bass_kwargs=None, **spmd_kwargs):
    w = np.asarray(w)
    Rm = _build_rotation_matrices(np.asarray(angles))
    # r_host[c, blk*B + c'] = R[blk][c, c']  (contiguous per SBUF partition c)
    r_host = np.ascontiguousarray(Rm.transpose(1, 0, 2)).reshape(B, NB * B)
    nc = _build_bass(mm_f32r=mm_f32r, **(bass_kwargs or {}))
    in_maps = [
        {"wt": w[i * ROWS : (i + 1) * ROWS, :].T, "r": r_host}
        for i in range(NCORES)
    ]
    res = run_bass_kernel_spmd(
        nc, in_maps, core_ids=list(range(NCORES)), trace=trace, **spmd_kwargs
    )
    out = np.empty((O, IN_F), dtype=np.float32)
    for i in range(NCORES):
        out[i * ROWS : (i + 1) * ROWS, :] = res.results[i]["out_t"].T
    return out, res


def kernel(w, angles):
    out, _ = kernel_impl(w, angles, trace=False)
    return out


# revision 15
# speedup vs baseline: 1.2187x; 1.0419x over previous
"""Trainium2 Bass kernel for BlockGivensRotation (w @ R, block-diagonal).

The reference applies, per 128-column block of w, 8 sequential sweeps of 127
adjacent-plane Givens rotations.  The composition of all 1016 rotations of a
block is a fixed 128x128 orthogonal matrix R_nb that depends only on `angles`,
so the whole op is `out[:, nb*128:(nb+1)*128] = w[:, nb*128:(nb+1)*128] @ R_nb`
- a block-diagonal matmul, ideal for the tensor engine.

Host side: compose R (tiny: 64x128x128, built in f64 from the 65K angles).
Device side: shard the 64 column-blocks across the 8 cores (8 blocks each) so
every core only needs its own slice of R (512 KB, not a 4 MB replica).  Each
core streams w.T tiles from DRAM, matmuls with the per-block stationary R in
full fp32, and writes out.T tiles back.  w is fed transposed so the
contraction dim (block columns) lies on SBUF partitions with fully contiguous
DMA; the host transposes shards in/out.

The kernel is DMA-bound (~64.5 MB of HBM I/O per core; the 8 cores together
sit at the chip HBM roofline), so the structure keeps the DMA queues
saturated: 2 MB loads/stores split across the two HWDGE rings (w loads on SP,
R loads and out stores on ACT), a halved first tile so the PE starts early,
and enough tile-pool depth that the PE never starves.
"""

import numpy as np

import concourse.bacc as bacc
import concourse.mybir as mybir
import concourse.tile as tile
from concourse.bass_utils import run_bass_kernel_spmd

O = 8192          # w rows
IN_F = 8192       # w cols
B = 128           # Givens block size
NB = IN_F // B    # 64 blocks
NCORES = 8
BPC = NB // NCORES  # 8 column-blocks per core
F32 = mybir.dt.float32


def _build_rotation_matrices(angles: np.ndarray) -> np.ndarray:
    """Compose the sweeps of adjacent Givens rotations into one 128x128
    matrix per block by applying the reference recurrence to the identity
    (in float64, rounded once to f32)."""
    nb, s, bm1 = angles.shape
    b = bm1 + 1
    ang = np.asarray(angles, dtype=np.float64)
    c = np.cos(ang)
    sn = np.sin(ang)
    R = np.broadcast_to(np.eye(b), (nb, b, b)).copy()  # [NB, basis row, col]
    for sweep in range(s):
        cs, ss = c[:, sweep, :], sn[:, sweep, :]
        carry = R[:, :, 0].copy()
        for i in range(bm1):
            col_j = R[:, :, i + 1]
            ci = cs[:, i][:, None]
            si = ss[:, i][:, None]
            R[:, :, i] = ci * carry - si * col_j
            carry = si * carry + ci * col_j
        R[:, :, b - 1] = carry
    return R.astype(np.float32)


def _build_bass(
    rows=O,
    bpc=BPC,
    ncores=NCORES,
    tile_rows=4096,
    wt_bufs=5,
    out_bufs=4,
    r_first=2,
    split_first=True,
):
    """Per-core program over this core's `bpc` column-blocks of w:

        out_t[blk*B + c', r] = sum_c R[blk][c, c'] * wt[blk*B + c, r]

    rows: w rows (full, 8192); tile_rows: rows per DMA tile;
    wt_bufs/out_bufs: pipeline depth; r_first: blocks of R in the first
    (small) R chunk so the first matmul isn't gated on the whole R slice;
    split_first: halve the first w tile so the PE starts sooner.
    """
    nc = bacc.Bacc(
        "TRN2", target_bir_lowering=False, debug=False, num_devices=ncores
    )
    wt = nc.dram_tensor("wt", [bpc * B, rows], F32, kind="ExternalInput")
    r = nc.dram_tensor("r", [B, bpc * B], F32, kind="ExternalInput")
    out_t = nc.dram_tensor("out_t", [bpc * B, rows], F32, kind="ExternalOutput")

    hs = 512                    # moving free-dim per matmul (fp32 max 512)

    with tile.TileContext(nc) as tc:
        with (
            tc.tile_pool(name="rp", bufs=1) as rp,
            tc.tile_pool(name="wtp", bufs=wt_bufs) as wtp,
            tc.tile_pool(name="outp", bufs=out_bufs) as outp,
            tc.tile_pool(name="psp", bufs=8, space="PSUM") as psp,
        ):
            # This core's R slice (bpc*64KB), in two chunks on the ACT ring
            # so it transfers in parallel with the first w tile on SP.
            rf = min(r_first, bpc)
            r_a = rp.tile([B, rf * B], F32, tag="ra")
            nc.scalar.dma_start(r_a[:], r[:, : rf * B])
            r_b = None
            if rf < bpc:
                r_b = rp.tile([B, (bpc - rf) * B], F32, tag="rb")
                nc.scalar.dma_start(r_b[:], r[:, rf * B :])
            for blk in range(bpc):
                if blk < rf:
                    r_ap = r_a[:, blk * B : (blk + 1) * B]
                else:
                    r_ap = r_b[:, (blk - rf) * B : (blk - rf + 1) * B]
                segs = [
                    (o, min(tile_rows, rows - o)) for o in range(0, rows, tile_rows)
                ]
                if split_first and blk == 0 and tile_rows >= 1024:
                    half = tile_rows // 2
                    segs = [(0, half), (half, half)] + segs[1:]
                for o, seg in segs:
                    wt_tile = wtp.tile([B, seg], F32, tag="wt")
                    nc.sync.dma_start(
                        wt_tile[:], wt[blk * B : (blk + 1) * B, o : o + seg]
                    )
                    out_tile = outp.tile([B, seg], F32, tag="out")
                    for h in range(seg // hs):
                        ps = psp.tile([B, hs], F32)
                        nc.tensor.matmul(
                            ps[:],
                            r_ap,
                            wt_tile[:, h * hs : (h + 1) * hs],
                            start=True,
                            stop=True,
                        )
                        nc.vector.tensor_copy(
                            out_tile[:, h * hs : (h + 1) * hs], ps[:]
                        )
                    # out-stores ride the second HWDGE ring (ACT)
                    nc.scalar.dma_start(
                        out_t[blk * B : (blk + 1) * B, o : o + seg], out_tile[:]
                    )
    nc.compile()
    return nc


def kernel_impl(w, angles, trace=False, bass_kwargs=None, **spmd_kwargs):
    w = np.asarray(w)
    Rm = _build_rotation_matrices(np.asarray(angles))
    # r_host[c, blk*B + c'] = R[blk][c, c']  (contiguous per SBUF partition c)
    r_host = np.ascontiguousarray(Rm.transpose(1, 0, 2)).reshape(B, NB * B)
    nc = _build_bass(**(bass_kwargs or {}))
    csz = BPC * B  # 1024 w-columns per core
    in_maps = [
        {
            "wt": w[:, i * csz : (i + 1) * csz].T,
            "r": r_host[:, i * csz : (i + 1) * csz],
        }
        for i in range(NCORES)
    ]
    res = run_bass_kernel_spmd(
        nc, in_maps, core_ids=list(range(NCORES)), trace=trace, **spmd_kwargs
    )
    out = np.empty((O, IN_F), dtype=np.float32)
    for i in range(NCORES):
        out[:, i * csz : (i + 1) * csz] = res.results[i]["out_t"].T
    return out, res


def kernel(w, angles):
    out, _ = kernel_impl(w, angles, trace=False)
    return out


# revision 18
# speedup vs baseline: 1.2607x; 1.0344x over previous
"""Trainium2 Bass kernel for BlockGivensRotation (w @ R, block-diagonal).

The reference applies, per 128-column block of w, 8 sequential sweeps of 127
adjacent-plane Givens rotations.  The composition of all 1016 rotations of a
block is a fixed 128x128 orthogonal matrix R_nb that depends only on `angles`,
so the whole op is `out[:, nb*128:(nb+1)*128] = w[:, nb*128:(nb+1)*128] @ R_nb`
- a block-diagonal matmul, ideal for the tensor engine.

Host side: compose R (tiny: 64x128x128, built in f64 from the 65K angles).
Device side: shard the 64 column-blocks across the 8 cores (8 blocks each) so
every core only needs its own slice of R (512 KB, not a 4 MB replica).  Each
core streams w.T tiles from DRAM, matmuls with the per-block stationary R in
full fp32, and writes out.T tiles back.  w is fed transposed so the
contraction dim (block columns) lies on SBUF partitions with fully contiguous
DMA; the host transposes shards in/out.

The kernel is DMA-bound (~64.5 MB of HBM I/O per core; the 8 cores together
sit at the chip HBM roofline), so the structure keeps the DMA queues
saturated: 2 MB loads/stores split across the two HWDGE rings (w loads on SP,
R loads and out stores on ACT), a halved first tile so the PE starts early,
and enough tile-pool depth that the PE never starves.
"""

import numpy as np

import concourse.bacc as bacc
import concourse.mybir as mybir
import concourse.tile as tile
from concourse.bass_utils import run_bass_kernel_spmd

O = 8192          # w rows
IN_F = 8192       # w cols
B = 128           # Givens block size
NB = IN_F // B    # 64 blocks
NCORES = 8
BPC = NB // NCORES  # 8 column-blocks per core
F32 = mybir.dt.float32


def _build_rotation_matrices(angles: np.ndarray) -> np.ndarray:
    """Compose the sweeps of adjacent Givens rotations into one 128x128
    matrix per block by applying the reference recurrence to the identity
    (in float64, rounded once to f32)."""
    nb, s, bm1 = angles.shape
    b = bm1 + 1
    ang = np.asarray(angles, dtype=np.float64)
    c = np.cos(ang)
    sn = np.sin(ang)
    R = np.broadcast_to(np.eye(b), (nb, b, b)).copy()  # [NB, basis row, col]
    for sweep in range(s):
        cs, ss = c[:, sweep, :], sn[:, sweep, :]
        carry = R[:, :, 0].copy()
        for i in range(bm1):
            col_j = R[:, :, i + 1]
            ci = cs[:, i][:, None]
            si = ss[:, i][:, None]
            R[:, :, i] = ci * carry - si * col_j
            carry = si * carry + ci * col_j
        R[:, :, b - 1] = carry
    return R.astype(np.float32)


def _build_bass(
    rows=O,
    bpc=BPC,
    ncores=NCORES,
    tile_rows=4096,
    wt_bufs=5,
    out_bufs=4,
    r_first=2,
    split_first=True,
):
    """Per-core program over this core's `bpc` column-blocks of w:

        out_t[blk*B + c', r] = sum_c R[blk][c, c'] * wt[blk*B + c, r]

    rows: w rows (full, 8192); tile_rows: rows per DMA tile;
    wt_bufs/out_bufs: pipeline depth; r_first: blocks of R in the first
    (small) R chunk so the first matmul isn't gated on the whole R slice;
    split_first: halve the first w tile so the PE starts sooner.
    """
    nc = bacc.Bacc(
        "TRN2", target_bir_lowering=False, debug=False, num_devices=ncores
    )
    wt = nc.dram_tensor("wt", [bpc * B, rows], F32, kind="ExternalInput")
    r = nc.dram_tensor("r", [B, bpc * B], F32, kind="ExternalInput")
    out_t = nc.dram_tensor("out_t", [bpc * B, rows], F32, kind="ExternalOutput")

    hs = 512                    # moving free-dim per matmul (fp32 max 512)

    with tile.TileContext(nc) as tc:
        with (
            tc.tile_pool(name="rp", bufs=1) as rp,
            tc.tile_pool(name="wtp", bufs=wt_bufs) as wtp,
            tc.tile_pool(name="outp", bufs=out_bufs) as outp,
            tc.tile_pool(name="psp", bufs=8, space="PSUM") as psp,
        ):
            # This core's R slice (bpc*64KB), in two chunks on the ACT ring
            # so it transfers in parallel with the first w tile on SP.
            rf = min(r_first, bpc)
            r_a = rp.tile([B, rf * B], F32, tag="ra")
            nc.scalar.dma_start(r_a[:], r[:, : rf * B])
            r_b = None
            if rf < bpc:
                r_b = rp.tile([B, (bpc - rf) * B], F32, tag="rb")
                nc.scalar.dma_start(r_b[:], r[:, rf * B :])
            for blk in range(bpc):
                if blk < rf:
                    r_ap = r_a[:, blk * B : (blk + 1) * B]
                else:
                    r_ap = r_b[:, (blk - rf) * B : (blk - rf + 1) * B]
                segs = [
                    (o, min(tile_rows, rows - o)) for o in range(0, rows, tile_rows)
                ]
                if split_first and blk == 0 and tile_rows >= 1024:
                    half = tile_rows // 2
                    segs = [(0, half), (half, half)] + segs[1:]
                for o, seg in segs:
                    wt_tile = wtp.tile([B, seg], F32, tag="wt")
                    nc.sync.dma_start(
                        wt_tile[:], wt[blk * B : (blk + 1) * B, o : o + seg]
                    )
                    out_tile = outp.tile([B, seg], F32, tag="out")
                    for h in range(seg // hs):
                        ps = psp.tile([B, hs], F32)
                        nc.tensor.matmul(
                            ps[:],
                            r_ap,
                            wt_tile[:, h * hs : (h + 1) * hs],
                            start=True,
                            stop=True,
                        )
                        nc.vector.tensor_copy(
                            out_tile[:, h * hs : (h + 1) * hs], ps[:]
                        )
                    # out-stores ride the second HWDGE ring (ACT)
                    nc.scalar.dma_start(
                        out_t[blk * B : (blk + 1) * B, o : o + seg], out_tile[:]
                    )
    nc.compile()
    return nc


def kernel_impl(w, angles, trace=False, bass_kwargs=None, **spmd_kwargs):
    w = np.asarray(w)
    Rm = _build_rotation_matrices(np.asarray(angles))
    # r_host[c, blk*B + c'] = R[blk][c, c']  (contiguous per SBUF partition c)
    r_host = np.ascontiguousarray(Rm.transpose(1, 0, 2)).reshape(B, NB * B)
    nc = _build_bass(**(bass_kwargs or {}))
    csz = BPC * B  # 1024 w-columns per core
    in_maps = [
        {
            "wt": w[:, i * csz : (i + 1) * csz].T,
            "r": r_host[:, i * csz : (i + 1) * csz],
        }
        for i in range(NCORES)
    ]
    res = run_bass_kernel_spmd(
        nc, in_maps, core_ids=list(range(NCORES)), trace=trace, **spmd_kwargs
    )
    out = np.empty((O, IN_F), dtype=np.float32)
    for i in range(NCORES):
        out[:, i * csz : (i + 1) * csz] = res.results[i]["out_t"].T
    return out, res


def kernel(w, angles):
    out, _ = kernel_impl(w, angles, trace=False)
    return out


# revision 20
# speedup vs baseline: 1.2694x; 1.0069x over previous
"""Trainium2 Bass kernel for BlockGivensRotation (w @ R, block-diagonal).

The reference applies, per 128-column block of w, 8 sequential sweeps of 127
adjacent-plane Givens rotations.  The composition of all 1016 rotations of a
block is a fixed 128x128 orthogonal matrix R_nb that depends only on `angles`,
so the whole op is `out[:, nb*128:(nb+1)*128] = w[:, nb*128:(nb+1)*128] @ R_nb`
- a block-diagonal matmul, ideal for the tensor engine.

Host side: compose R (tiny: 64x128x128, built in f64 from the 65K angles).
Device side: shard the 64 column-blocks across the 8 cores (8 blocks each) so
every core only needs its own slice of R (512 KB, not a 4 MB replica).  Each
core streams w.T tiles from DRAM, matmuls with the per-block stationary R in
full fp32, and writes out.T tiles back.  w is fed transposed so the
contraction dim (block columns) lies on SBUF partitions with fully contiguous
DMA; the host transposes shards in/out.

The kernel is DMA-bound (~64.5 MB of HBM I/O per core; the 8 cores together
sit at the chip HBM roofline), so the structure keeps the DMA queues
saturated: 2 MB loads/stores split across the two HWDGE rings (w loads on SP,
R loads and out stores on ACT), a halved first tile so the PE starts early,
and enough tile-pool depth that the PE never starves.
"""

import numpy as np

import concourse.bacc as bacc
import concourse.mybir as mybir
import concourse.tile as tile
from concourse.bass_utils import run_bass_kernel_spmd

O = 8192          # w rows
IN_F = 8192       # w cols
B = 128           # Givens block size
NB = IN_F // B    # 64 blocks
NCORES = 8
BPC = NB // NCORES  # 8 column-blocks per core
F32 = mybir.dt.float32


def _build_rotation_matrices(angles: np.ndarray) -> np.ndarray:
    """Compose the sweeps of adjacent Givens rotations into one 128x128
    matrix per block by applying the reference recurrence to the identity
    (in float64, rounded once to f32)."""
    nb, s, bm1 = angles.shape
    b = bm1 + 1
    ang = np.asarray(angles, dtype=np.float64)
    c = np.cos(ang)
    sn = np.sin(ang)
    R = np.broadcast_to(np.eye(b), (nb, b, b)).copy()  # [NB, basis row, col]
    for sweep in range(s):
        cs, ss = c[:, sweep, :], sn[:, sweep, :]
        carry = R[:, :, 0].copy()
        for i in range(bm1):
            col_j = R[:, :, i + 1]
            ci = cs[:, i][:, None]
            si = ss[:, i][:, None]
            R[:, :, i] = ci * carry - si * col_j
            carry = si * carry + ci * col_j
        R[:, :, b - 1] = carry
    return R.astype(np.float32)


def _build_bass(
    rows=O,
    bpc=BPC,
    ncores=NCORES,
    tile_rows=4096,
    wt_bufs=5,
    out_bufs=4,
    r_first=2,
    split_first=True,
):
    """Per-core program over this core's `bpc` column-blocks of w:

        out_t[blk*B + c', r] = sum_c R[blk][c, c'] * wt[blk*B + c, r]

    rows: w rows (full, 8192); tile_rows: rows per DMA tile;
    wt_bufs/out_bufs: pipeline depth; r_first: blocks of R in the first
    (small) R chunk so the first matmul isn't gated on the whole R slice;
    split_first: halve the first w tile so the PE starts sooner.
    """
    nc = bacc.Bacc(
        "TRN2", target_bir_lowering=False, debug=False, num_devices=ncores
    )
    wt = nc.dram_tensor("wt", [bpc * B, rows], F32, kind="ExternalInput")
    r = nc.dram_tensor("r", [B, bpc * B], F32, kind="ExternalInput")
    out_t = nc.dram_tensor("out_t", [bpc * B, rows], F32, kind="ExternalOutput")

    hs = 512                    # moving free-dim per matmul (fp32 max 512)

    with tile.TileContext(nc) as tc:
        with (
            tc.tile_pool(name="rp", bufs=1) as rp,
            tc.tile_pool(name="wtp", bufs=wt_bufs) as wtp,
            tc.tile_pool(name="outp", bufs=out_bufs) as outp,
            tc.tile_pool(name="psp", bufs=8, space="PSUM") as psp,
        ):
            # This core's R slice (bpc*64KB), in two chunks on the ACT ring
            # so it transfers in parallel with the first w tile on SP.
            rf = min(r_first, bpc)
            r_a = rp.tile([B, rf * B], F32, tag="ra")
            nc.scalar.dma_start(r_a[:], r[:, : rf * B])
            r_b = None
            if rf < bpc:
                r_b = rp.tile([B, (bpc - rf) * B], F32, tag="rb")
                nc.scalar.dma_start(r_b[:], r[:, rf * B :])
            for blk in range(bpc):
                if blk < rf:
                    r_ap = r_a[:, blk * B : (blk + 1) * B]
                else:
                    r_ap = r_b[:, (blk - rf) * B : (blk - rf + 1) * B]
                segs = [
                    (o, min(tile_rows, rows - o)) for o in range(0, rows, tile_rows)
                ]
                if split_first and blk == 0 and tile_rows >= 1024:
                    half = tile_rows // 2
                    segs = [(0, half), (half, half)] + segs[1:]
                for o, seg in segs:
                    wt_tile = wtp.tile([B, seg], F32, tag="wt")
                    nc.sync.dma_start(
                        wt_tile[:], wt[blk * B : (blk + 1) * B, o : o + seg]
                    )
                    out_tile = outp.tile([B, seg], F32, tag="out")
                    for h in range(seg // hs):
                        ps = psp.tile([B, hs], F32)
                        nc.tensor.matmul(
                            ps[:],
                            r_ap,
                            wt_tile[:, h * hs : (h + 1) * hs],
                            start=True,
                            stop=True,
                        )
                        nc.vector.tensor_copy(
                            out_tile[:, h * hs : (h + 1) * hs], ps[:]
                        )
                    # out-stores ride the second HWDGE ring (ACT)
                    nc.scalar.dma_start(
                        out_t[blk * B : (blk + 1) * B, o : o + seg], out_tile[:]
                    )
    nc.compile()
    return nc


def kernel_impl(w, angles, trace=False, bass_kwargs=None, **spmd_kwargs):
    w = np.asarray(w)
    Rm = _build_rotation_matrices(np.asarray(angles))
    # r_host[c, blk*B + c'] = R[blk][c, c']  (contiguous per SBUF partition c)
    r_host = np.ascontiguousarray(Rm.transpose(1, 0, 2)).reshape(B, NB * B)
    nc = _build_bass(**(bass_kwargs or {}))
    csz = BPC * B  # 1024 w-columns per core
    in_maps = [
        {
            "wt": w[:, i * csz : (i + 1) * csz].T,
            "r": r_host[:, i * csz : (i + 1) * csz],
        }
        for i in range(NCORES)
    ]
    res = run_bass_kernel_spmd(
        nc, in_maps, core_ids=list(range(NCORES)), trace=trace, **spmd_kwargs
    )
    out = np.empty((O, IN_F), dtype=np.float32)
    for i in range(NCORES):
        out[:, i * csz : (i + 1) * csz] = res.results[i]["out_t"].T
    return out, res


def kernel(w, angles):
    out, _ = kernel_impl(w, angles, trace=False)
    return out
